# revision 94
# baseline (speedup 1.0000x reference)
"""Local (bucketed) attention Bass kernel for Trainium2, 8 NeuronCores SPMD.

Problem (hardcoded): B=8, H=8, T=8192, E=64, BUCKETS=128, bucket=64,
look_backward=1, look_forward=0, causal, no 1/sqrt(E) scaling.

Sharding: batch*heads (64) split across 8 cores -> 8 bh per core.
Each core processes its 8 bh as 4 "pairs"; within a pair, bh 2p lives on
SBUF partitions 0..63 ("stream A") and bh 2p+1 on partitions 64..127
("stream B") so every op runs at full 128-partition width.

Math per (bh, window w): keys/values = buckets {w-1, w}.
  dotsT[j, i] = sum_e k[key_bucket*64+j, e] * q[w*64+i, e]   (transposed!)
  exp -> bf16, causal tri-mask on the "cur" (key==w) half,
  out[i, :64+1] = sum_j expT[j, i] * v_aug[j, :]  accumulated over the
  prev and cur halves in PSUM; column 64 of v_aug is ones => row sums.
  out = out[:, :64] * (1 / out[:, 64]).

MM1 (mode "fp16bd"): q, k shipped fp16 (same bytes as bf16, 8x the
mantissa). For pairs >= 1 a block-diagonal stationary kt_bd[:, c, :]
(stream A: E rows 0:64 -> key cols 0:64, stream B: E rows 64:128 ->
cols 64:128, zeros elsewhere) lets ONE 128-partition matmul per key
bucket produce both streams' dotsT; the PE charges moving columns only,
so this halves mm1 PE time vs two 64-partition matmuls. kt_bd lives in
two persistent buffers, zero-filled once (off-diagonal blocks never
rewritten); per pair the diagonal strips are copied in (A halves on
vector, B halves on gpsimd). Pair 0 uses the plain two-matmul path so
PE can start before any zero-fill/build completes.

Host-side prep (free vs. HW exec time): q, k pre-transposed to [E, T]
per bh and cast fp16; v augmented with a ones column, cast bf16.
Output shipped bf16, upcast on host.
"""

import numpy as np
import ml_dtypes

BH_PER_CORE = 8
N_CORES = 8
T = 8192
E = 64
BS = 64  # bucket size
NBUCK = T // BS  # 128

MM1_MODE = "fp16bd"
DENSE_KTBD0 = False  # ship pair 0's kt_bd dense (+2 MiB DMA, -6.8us PE)

_PROGRAM_CACHE = {}


def _build_program(mm1_mode, n_pairs=BH_PER_CORE // 2, nbuck=NBUCK):
    import concourse.bass as bass
    import concourse.tile as tile
    from concourse import bacc, mybir

    F32 = mybir.dt.float32
    F32R = mybir.dt.float32r
    BF16 = mybir.dt.bfloat16
    FP16 = mybir.dt.float16
    Exp = mybir.ActivationFunctionType.Exp
    mult = mybir.AluOpType.mult

    qk_dt = {"fp32r": F32R, "fp16": FP16, "fp16bd": FP16}.get(mm1_mode, BF16)
    qk_hilo = mm1_mode == "bf16x2"
    blockdiag = mm1_mode == "fp16bd"

    nc = bacc.Bacc("TRN2", target_bir_lowering=False, debug=False,
                   num_devices=N_CORES)

    seqlen = nbuck * BS
    qk_shape = [n_pairs, 128, 2, seqlen] if qk_hilo else [n_pairs, 128, seqlen]
    qt_d = nc.dram_tensor("qt", qk_shape, qk_dt, kind="ExternalInput").ap()
    kt_d = nc.dram_tensor("kt", qk_shape, qk_dt, kind="ExternalInput").ap()
    va_d = nc.dram_tensor("va", [n_pairs, 128, nbuck, BS + 1], BF16,
                          kind="ExternalInput").ap()
    ktbd0_d = None
    if blockdiag and DENSE_KTBD0:
        # pair 0's block-diagonal stationary shipped dense (zeros included):
        # its on-chip build could not be hidden behind a previous pair.
        ktbd0_d = nc.dram_tensor("ktbd0", [128, nbuck, 128], qk_dt,
                                 kind="ExternalInput").ap()
    out_d = nc.dram_tensor("out", [n_pairs, 128, nbuck, BS], BF16,
                           kind="ExternalOutput").ap()

    # Causal tri mask for the "cur" half, both streams: keep iff i >= j.
    mask_np = (np.arange(BS)[None, :] >= np.arange(BS)[:, None]).astype(
        ml_dtypes.bfloat16)
    mask_np = np.concatenate([mask_np, mask_np], axis=0)  # [128, 64]
    mask_dram = nc.inline_tensor(np.ascontiguousarray(mask_np), name="trimask")

    SEXP = 16  # stationaries per exp-sbuf tile
    SPB = 8 if blockdiag else 4   # stationaries per PSUM fat tile
    NP = 7                        # windows per out-psum tile (7 per bank)
    NPB = 7                       # windows per PSUM bank (7*65*4 <= 2048)
    NBLK = 4 if mm1_mode == "fp32r" else 2  # moving blocks per mm1

    with tile.TileContext(nc) as tc:
        with (
            tc.tile_pool(name="consts", bufs=1) as consts,
            tc.tile_pool(name="qk", bufs=2) as qkp,
            tc.tile_pool(name="vap", bufs=2) as vap,
            tc.tile_pool(name="expp", bufs=3) as expp,
            tc.tile_pool(name="outsb", bufs=8) as outsbp,
            tc.tile_pool(name="rp", bufs=3) as rp,
            tc.tile_pool(name="fat", bufs=2 if blockdiag else 3,
                         space="PSUM") as fatp,
            tc.tile_pool(name="outps", bufs=4 if blockdiag else 2,
                         space="PSUM") as outpsp,
        ):
            bd_tiles = None
            if blockdiag:
                bd_tiles = [consts.tile([128, nbuck, 128], qk_dt,
                                        name=f"ktbd{i}", tag=f"ktbd{i}")
                            for i in range(2)]
            # (buf1's zero-fill is emitted after pair 0's load DMAs so the
            # Pool-issued mask/va transfers reach the DMA device first;
            # buf0 is zeroed in small act strips during pair 0's compute.)
            mask_sb = consts.tile([128, BS], BF16)

            def emit_pair_dmas(p):
                """Input DMAs. Pair 0 (h-split) uses small chunks so the
                first matmul can start ~3us in."""
                qk_tile_shape = [128, 2, seqlen] if qk_hilo else [128, seqlen]
                qt_sb = qkp.tile(qk_tile_shape, qk_dt, tag="qt")
                kt_sb = qkp.tile(qk_tile_shape, qk_dt, tag="kt")
                va_sb = vap.tile([128, nbuck, BS + 1], BF16, tag="va")
                if qk_hilo:
                    nc.sync.dma_start(kt_sb[:], kt_d[p])
                    nc.sync.dma_start(qt_sb[:], qt_d[p])
                    nc.sync.dma_start(va_sb[:], va_d[p])
                    return qt_sb, kt_sb, va_sb, None
                if p == 0 and blockdiag and DENSE_KTBD0:
                    # pair 0: dense kt_bd chunks + qt in lockstep, small
                    # chunks so PE starts ~3us in. No lean kt needed.
                    bdt0 = bd_tiles[0]
                    nseg, nvseg = 8, 4
                    cseg = seqlen // nseg
                    bseg = nbuck // nseg
                    for s in range(nseg):
                        sl = slice(s * cseg, (s + 1) * cseg)
                        bl = slice(s * bseg, (s + 1) * bseg)
                        nc.sync.dma_start(bdt0[:, bl, :], ktbd0_d[:, bl, :])
                        nc.sync.dma_start(qt_sb[:, sl], qt_d[p, :, sl])
                        if s == 0:
                            nc.sync.dma_start(mask_sb[:], mask_dram.ap())
                        if s % (nseg // nvseg) == 0:
                            v = s // (nseg // nvseg)
                            vs = slice(v * (nbuck // nvseg),
                                       (v + 1) * (nbuck // nvseg))
                            nc.sync.dma_start(va_sb[:, vs, :],
                                              va_d[p, :, vs, :])
                    return qt_sb, kt_sb, va_sb, bdt0
                elif p == 0:
                    # mask and the first va quarter are issued from the
                    # (idle) Pool queue so neither the SP kt/qt stream nor
                    # the act exp stream is delayed at startup.
                    nc.gpsimd.dma_start(mask_sb[:], mask_dram.ap())
                    nc.gpsimd.dma_start(va_sb[:, 0:nbuck // 4, :],
                                        va_d[p, :, 0:nbuck // 4, :])
                    nseg, nvseg = 8, 4
                    cseg = seqlen // nseg
                    for s in range(nseg):
                        sl = slice(s * cseg, (s + 1) * cseg)
                        nc.sync.dma_start(kt_sb[:, sl], kt_d[p, :, sl])
                        nc.sync.dma_start(qt_sb[:, sl], qt_d[p, :, sl])
                        if s % (nseg // nvseg) == 1 and s > 2:
                            # va lags the kt/qt frontier by one segment
                            v = s // (nseg // nvseg)
                            vs = slice(v * (nbuck // nvseg),
                                       (v + 1) * (nbuck // nvseg))
                            nc.sync.dma_start(va_sb[:, vs, :],
                                              va_d[p, :, vs, :])
                else:
                    # kt first (both halves): the kt_bd build is the pair's
                    # critical prefetch; qt/va are consumed later.
                    half, hb = seqlen // 2, nbuck // 2
                    for s in (0, 1):
                        sl = slice(s * half, (s + 1) * half)
                        nc.sync.dma_start(kt_sb[:, sl], kt_d[p, :, sl])
                    for s in (0, 1):
                        sl = slice(s * half, (s + 1) * half)
                        vs = slice(s * hb, (s + 1) * hb)
                        nc.sync.dma_start(qt_sb[:, sl], qt_d[p, :, sl])
                        nc.sync.dma_start(va_sb[:, vs, :], va_d[p, :, vs, :])
                bdt = bd_tiles[p % 2] if (blockdiag and p >= 1) else None
                return qt_sb, kt_sb, va_sb, bdt

            def emit_pair_builds(p, tiles, s):
                """kt_bd diagonal-strip build (segment s) for pair p >= 1.
                Emitted mid/late in pair p-1's compute so it sits behind
                that pair's masks in the in-order DVE stream (A half) and
                ahead of the final out-DMA waits on Pool (B half)."""
                qt_sb, kt_sb, va_sb, bdt = tiles
                if bdt is None:
                    return
                ktv = kt_sb[:].rearrange("e (b k) -> e b k", k=BS)
                hb = nbuck // 2
                bsl = slice(s * hb, (s + 1) * hb)
                nc.vector.tensor_copy(bdt[0:64, bsl, 0:64],
                                      ktv[0:64, bsl, :])
                nc.gpsimd.tensor_copy(bdt[64:128, bsl, 64:128],
                                      ktv[64:128, bsl, :])

            def emit_mm2(w, exp_tiles, va_sb, state, p):
                """mm2 + normalize/store for window w (batched in PSUM)."""
                if state["open"] is None:
                    raw = outpsp.tile([128, NP // NPB, 512], F32, tag="outps")
                    out_ps = raw[:, :, 0:NPB * (BS + 1)].rearrange(
                        "p b (w x) -> p b w x", x=BS + 1)
                    state["open"] = (out_ps, [])
                out_ps, wlist = state["open"]
                slot = len(wlist)
                sb, si = slot // NPB, slot % NPB
                wlist.append(w)
                cur_t, cur_s = exp_tiles[w]
                if w > 0:
                    prev_t, prev_s = exp_tiles[w - 1]
                for h in (0, 64):
                    if w > 0:
                        nc.tensor.matmul(
                            out_ps[h:h + 64, sb, si, :],
                            lhsT=prev_t[h:h + 64, prev_s, 1, :],
                            rhs=va_sb[h:h + 64, w - 1, :],
                            start=True, stop=False,
                        )
                    nc.tensor.matmul(
                        out_ps[h:h + 64, sb, si, :],
                        lhsT=cur_t[h:h + 64, cur_s, 0, :],
                        rhs=va_sb[h:h + 64, w, :],
                        start=(w == 0), stop=True,
                    )
                if len(wlist) == NP or w == nbuck - 1:
                    nw = len(wlist)
                    nb = (nw + NPB - 1) // NPB
                    r_sb = rp.tile([128, NP // NPB, NPB], F32, tag="r")
                    ob = outsbp.tile([128, NP, BS], BF16, tag="ob")
                    obv = ob[:].rearrange("p (b w) x -> p b w x", w=NPB)
                    # negative priority offset: the out chain is latency-
                    # tolerant (deep ob + psum rotation), so ready masks
                    # overtake recip/scale in the DVE stream
                    with tc.high_priority(offset=-192):
                        if nw % NPB == 0:
                            nc.vector.reciprocal(r_sb[:, 0:nb, :],
                                                 out_ps[:, 0:nb, :, BS])
                            nc.vector.tensor_tensor(
                                obv[:, 0:nb, :, :],
                                out_ps[:, 0:nb, :, 0:BS],
                                r_sb[:, 0:nb, :, None].to_broadcast(
                                    (128, nb, NPB, BS)),
                                mult,
                            )
                        else:
                            assert nb == 1
                            nc.vector.reciprocal(r_sb[:, 0, 0:nw],
                                                 out_ps[:, 0, 0:nw, BS])
                            nc.vector.tensor_tensor(
                                obv[:, 0, 0:nw, :],
                                out_ps[:, 0, 0:nw, 0:BS],
                                r_sb[:, 0, 0:nw, None].to_broadcast(
                                    (128, nw, BS)),
                                mult,
                            )
                    # final chunk of the final pair goes out via the idle SP
                    # queue to shorten the kernel tail. Out DMAs are
                    # latency-tolerant (deep ob rotation): deprioritize so
                    # kt_bd builds sharing the Pool stream overtake them.
                    last = (p == n_pairs - 1 and w == nbuck - 1)
                    eng = nc.sync if last else nc.gpsimd
                    with tc.high_priority(offset=-512):
                        eng.dma_start(
                            out_d[p, :, wlist[0]:wlist[0] + nw, :],
                            ob[:, 0:nw, :],
                        )
                    state["nchunk"] += 1
                    state["open"] = None

            # mm2 pipeline state carried ACROSS pairs: each pending entry is
            # (pair, window, exp_tiles, va_sb, state); the last batch of
            # pair p drains during pair p+1's first batch so PE never sees
            # a pipeline-fill bubble at pair boundaries.
            pipe = {"pending": []}
            last_exp = {"i": None}  # most recent exp act, for zstrip deps

            def drain_pending():
                for (pp, w, et, va, st) in pipe["pending"]:
                    emit_mm2(w, et, va, st, pp)
                pipe["pending"] = []

            def emit_pair_compute(p, tiles, callbacks=None):
                qt_sb, kt_sb, va_sb, bdt = tiles
                qt_mm = qt_sb[:]
                kt_mm = kt_sb[:]

                exp_tiles = {}
                state = {"open": None, "nchunk": 0}
                for w0 in range(0, nbuck, SEXP):
                    exp_sb = expp.tile([128, SEXP, 2, BS], BF16, tag="exp")
                    # first batch of pair 0: half-size fat tiles so the
                    # first exp->mask->mm2 chain starts ~0.5us earlier
                    spb = SPB // 2 if (p == 0 and w0 == 0) else SPB
                    for g0 in range(0, SEXP, spb):
                        fat = fatp.tile([128, spb, NBLK, BS], F32, tag="fat")
                        for j in range(spb):
                            c = w0 + g0 + j
                            nblk = min(NBLK, nbuck - c)
                            cs, ce = c * BS, (c + nblk) * BS
                            if bdt is not None:
                                nc.tensor.matmul(
                                    fat[:, j, 0:nblk, :],
                                    lhsT=bdt[:, c, :],
                                    rhs=qt_mm[:, cs:ce],
                                    start=True, stop=True,
                                )
                            elif qk_hilo:
                                for h in (0, 64):
                                    passes = [(0, 0, True, False),
                                              (1, 0, False, False),
                                              (0, 1, False, True)]
                                    for kh, qh_, st, sp in passes:
                                        nc.tensor.matmul(
                                            fat[h:h + 64, j, 0:nblk, :],
                                            lhsT=kt_mm[h:h + 64, kh,
                                                       cs:c * BS + BS],
                                            rhs=qt_mm[h:h + 64, qh_, cs:ce],
                                            start=st, stop=sp,
                                        )
                            else:
                                for h in (0, 64):
                                    nc.tensor.matmul(
                                        fat[h:h + 64, j, 0:nblk, :],
                                        lhsT=kt_mm[h:h + 64, cs:cs + BS],
                                        rhs=qt_mm[h:h + 64, cs:ce],
                                        start=True, stop=True,
                                    )
                            if nblk < 2:
                                # prev(c+1) does not exist (c == last bucket);
                                # fill so the batched exp reads finite data.
                                nc.vector.memset(fat[:, j, 1, :], 0.0)
                        # exp of both blocks of each stationary in this tile
                        last_exp["i"] = nc.scalar.activation(
                            exp_sb[:, g0:g0 + spb, :, :],
                            fat[:, :, 0:2, :],
                            Exp,
                        )
                        # causal tri mask on this tile's cur blocks right
                        # away (finer grain -> lower exp-to-mm2 latency);
                        # high priority: mm2 stalls on it, unlike the
                        # recip/scale work sharing the DVE stream
                        with tc.high_priority(offset=64):
                            nc.vector.tensor_tensor(
                                exp_sb[:, g0:g0 + spb, 0, :],
                                exp_sb[:, g0:g0 + spb, 0, :],
                                mask_sb[:, None, :].to_broadcast(
                                    (128, spb, BS)),
                                mult,
                            )
                    for s in range(w0, w0 + SEXP):
                        exp_tiles[s] = (exp_sb, s - w0)

                    # software pipeline: run mm2 for the PREVIOUS batch now
                    # (possibly the previous pair's last batch), so PE always
                    # has mm1 work queued ahead of mm2 stalls.
                    drain_pending()
                    for s in list(exp_tiles):
                        if s < w0 - 1:
                            del exp_tiles[s]
                    pipe["pending"] = [(p, w, exp_tiles, va_sb, state)
                                       for w in range(w0, w0 + SEXP)]
                    if callbacks and w0 // SEXP in callbacks:
                        callbacks[w0 // SEXP]()
                if p == n_pairs - 1:
                    drain_pending()

            # pair-level software pipeline: pair p+1's input DMAs are
            # emitted two batches into pair p's compute (keeps the DMA
            # device fed); its kt_bd builds are emitted after the last
            # batch's mask, behind everything latency-critical.
            tiles = {0: emit_pair_dmas(0)}
            if blockdiag:
                # zero-fill buf1 on the still mostly idle Pool engine;
                # off-diagonal blocks stay zero forever (per-pair builds
                # only rewrite the diagonal strips).
                nc.gpsimd.memset(bd_tiles[1][:], 0.0)
            nbatch = nbuck // SEXP

            def make_cbs(p):
                cbs = {}
                steps = []
                if p + 1 < n_pairs:
                    steps.append((2, lambda np_=p + 1: tiles.__setitem__(
                        np_, emit_pair_dmas(np_))))
                    steps.append((4, lambda np_=p + 1: emit_pair_builds(
                        np_, tiles[np_], 0)))
                    steps.append((6, lambda np_=p + 1: emit_pair_builds(
                        np_, tiles[np_], 1)))
                if (p == 0 and blockdiag and not DENSE_KTBD0
                        and n_pairs > 2):
                    # zero buf0 in 16 strips, two per batch, on act slack
                    # (first needed by pair 2's mm1)
                    def zstrip(k):
                        st = k * (nbuck // 16)
                        with tc.high_priority(offset=-256):
                            zi = nc.scalar.memzero(
                                bd_tiles[0][:, st:st + nbuck // 16, :])
                        # (chaining strips behind exps via add_dep_helper
                        # removes the startup act race but nets out slower:
                        # the freed PE time just hits the DMA-arrival wall
                        # at the next pair boundary instead.)
                        del zi
                    # skip batch 0: a strip dispatched just before the first
                    # exp becomes ready would delay the whole startup chain
                    k = 0
                    for b in range(1, nbatch):
                        n = 2 if b < nbatch - 1 else 16 - k
                        for _ in range(n):
                            steps.append((b, lambda k=k: zstrip(k)))
                            k += 1
                for b, f in steps:
                    cbs.setdefault(b, []).append(f)
                return {b: (lambda fs=fs: [f() for f in fs])
                        for b, fs in cbs.items()}

            for p in range(n_pairs):
                emit_pair_compute(p, tiles.pop(p), callbacks=make_cbs(p))

    nc.compile()
    return nc


def _get_program(mm1_mode=MM1_MODE):
    key = mm1_mode
    if key not in _PROGRAM_CACHE:
        _PROGRAM_CACHE[key] = _build_program(mm1_mode)
    return _PROGRAM_CACHE[key]


def _hilo(x):
    hi = x.astype(ml_dtypes.bfloat16)
    lo = (x - hi.astype(np.float32)).astype(ml_dtypes.bfloat16)
    return hi, lo


def _prep_core_inputs(qf, kf, vf, core, mm1_mode, n_pairs=BH_PER_CORE // 2):
    """qf,kf,vf: [64, T, E] float32 (bh-merged). Returns the core's in_map."""
    qk_np_dt = {"fp32r": np.float32, "fp16": np.float16,
                "fp16bd": np.float16}.get(mm1_mode, ml_dtypes.bfloat16)
    hilo = mm1_mode == "bf16x2"
    bh0 = core * BH_PER_CORE
    qk_shape = (n_pairs, 128, 2, T) if hilo else (n_pairs, 128, T)
    qt = np.empty(qk_shape, dtype=qk_np_dt)
    kt = np.empty(qk_shape, dtype=qk_np_dt)
    va = np.empty((n_pairs, 128, NBUCK, BS + 1), dtype=ml_dtypes.bfloat16)
    for p in range(n_pairs):
        a, b = bh0 + 2 * p, bh0 + 2 * p + 1
        if hilo:
            for half, bh in ((0, a), (1, b)):
                qh, ql = _hilo(qf[bh].T)
                kh, kl = _hilo(kf[bh].T)
                qt[p, half * 64:half * 64 + 64, 0] = qh
                qt[p, half * 64:half * 64 + 64, 1] = ql
                kt[p, half * 64:half * 64 + 64, 0] = kh
                kt[p, half * 64:half * 64 + 64, 1] = kl
        else:
            qt[p, 0:64] = qf[a].T
            qt[p, 64:128] = qf[b].T
            kt[p, 0:64] = kf[a].T
            kt[p, 64:128] = kf[b].T
        # v rows (bucket t, offset w) -> partition w, slot t
        va[p, 0:64, :, 0:64] = vf[a].reshape(NBUCK, BS, E).transpose(1, 0, 2)
        va[p, 64:128, :, 0:64] = vf[b].reshape(NBUCK, BS, E).transpose(1, 0, 2)
    va[..., 64] = 1.0
    out = {"qt": qt, "kt": kt, "va": va}
    if mm1_mode == "fp16bd" and DENSE_KTBD0:
        # pair 0's block-diagonal stationary, shipped dense
        ktbd0 = np.zeros((128, NBUCK, 128), dtype=qk_np_dt)
        ktbd0[0:64, :, 0:64] = kt[0, 0:64].reshape(64, NBUCK, BS)
        ktbd0[64:128, :, 64:128] = kt[0, 64:128].reshape(64, NBUCK, BS)
        out["ktbd0"] = ktbd0
    return out


def _unpack_out(res_out, core, out_full):
    """res_out: [4, 128, NBUCK, BS] -> writes into out_full [64, T, E]."""
    bh0 = core * BH_PER_CORE
    for p in range(res_out.shape[0]):
        a, b = bh0 + 2 * p, bh0 + 2 * p + 1
        # [i, bucket, e] -> [bucket, i, e] -> [T, e]
        out_full[a] = res_out[p, 0:64].transpose(1, 0, 2).reshape(T, E)
        out_full[b] = res_out[p, 64:128].transpose(1, 0, 2).reshape(T, E)


def kernel(q, k, v):
    from concourse.bass_utils import run_bass_kernel_spmd

    q = np.asarray(q, dtype=np.float32)
    k = np.asarray(k, dtype=np.float32)
    v = np.asarray(v, dtype=np.float32)
    Bq, Hq = q.shape[0], q.shape[1]
    qf = q.reshape(Bq * Hq, T, E)
    kf = k.reshape(Bq * Hq, T, E)
    vf = v.reshape(Bq * Hq, T, E)

    nc = _get_program(MM1_MODE)
    in_maps = [_prep_core_inputs(qf, kf, vf, c, MM1_MODE)
               for c in range(N_CORES)]
    res = run_bass_kernel_spmd(nc, in_maps, list(range(N_CORES)))

    out_full = np.empty((Bq * Hq, T, E), dtype=np.float32)
    for c in range(N_CORES):
        _unpack_out(np.asarray(res.results[c]["out"], dtype=np.float32), c,
                    out_full)
    return out_full.reshape(Bq, Hq, T, E)
